# revision 16
# baseline (speedup 1.0000x reference)
"""Trainium2 Bass kernel for nn_BaseCompressor2 (truncated one-pole IIR
compressor), v2.

Mapping: 32 batches / 8 cores = 4 per core. Partition p = (b, tb) with
b = p>>5 the batch and tb = p&31 a time block; each partition owns 8192
consecutive samples, processed as 4 chunks of 2048 columns.

Per chunk: squares on Act (bf16 out) -> e = sq0+sq1 on the otherwise-idle
PE via bf16 identity matmuls accumulating in PSUM (half tiles, double
buffered) -> DVE tensor_tensor_scan (fp32 state) reads PSUM -> gain chain:
x = ln(escale*y + eps) [Act]; u = relu(x+uk), v' = sqrtA*min(u,knee),
d = relu(x+ukk) as DVE tensor_scalar 2x ops; t = v'^2 [Act];
h = I@t + diag(negc1)@d on PE (PSUM); gain = exp(-h) [Act];
out_c = gain*s_c on Pool (gpsimd). The piecewise knee is replaced by the
algebraically equivalent h = A*min(u,knee)^2 + negc1*relu(u-knee) (max
deviation |c1|*5e-4, far below tolerance).

Cross-partition IIR carry: C[p] = y[p-1, 8191] via a tiny SBUF->SBUF
partition-shift DMA after the last scan, applied as y0[:, :T0] += pw*C
with one small STT (alpha^(t+1) underflows beyond T0 <= 512). Chunk 0's
gain chain is split into cols [512:2048] (carry-free, runs early) and
[0:512] (after the fix, tiny tail).

A manual InstLoadActFuncSet(natural_log_exp_and_others) removes all
mid-kernel activation-table reloads.
"""

import numpy as np

N, C, L = 32, 2, 262144
NCORES = 8
BPC = N // NCORES          # batches per core
P = 128
TBS = P // BPC             # 32 time blocks per batch
SPAN = L // TBS            # 8192 cols per partition
NCH = 4                    # chunks per core
CW = SPAN // NCH           # 2048 chunk width
HW = CW // 2               # 1024 scan/e half width
T0MAX = 512

# pcols layout
ALPHA, ESCALE, UK, UKK, KNEE, RTA, NEGC1 = range(7)
NPC = 7

_cache = {}


def _host_params(z_alpha_pre, log_threshold, log_ratio, log_knee):
    z = z_alpha_pre.astype(np.float64).reshape(-1)
    thr = log_threshold.astype(np.float64).reshape(-1) - 6.0
    knee = np.exp(log_knee.astype(np.float64).reshape(-1))
    r001 = 1.0 + np.exp(log_ratio.astype(np.float64).reshape(-1)) + 0.001
    alpha = 1.0 / (1.0 + np.exp(-z))
    negc1 = 1.0 - 1.0 / r001
    # infinite-IIR == 16384-tap truncation in f32
    assert np.max(16384.0 * np.log(alpha)) < -60.0, "alpha too close to 1"
    lna_max = np.max(np.log(alpha))
    t0 = int(np.ceil(24.0 / max(1e-9, -lna_max))) + 24
    t0 = min(T0MAX, max(64, t0))
    assert (t0 + 1) * lna_max < -23.0, "carry horizon exceeds 512 cols"
    vals = np.zeros((N, NPC), dtype=np.float64)
    vals[:, ALPHA] = alpha
    vals[:, ESCALE] = (1.0 - alpha) / 2.0
    vals[:, UK] = knee / 2.0 - thr
    vals[:, UKK] = -knee / 2.0 - thr
    vals[:, KNEE] = knee
    vals[:, RTA] = np.sqrt(negc1 / (2.0 * (knee + 0.001)))
    vals[:, NEGC1] = negc1
    return vals.astype(np.float32), negc1, alpha, t0


def _build_program(T0):
    from contextlib import ExitStack

    import concourse.bacc as bacc
    import concourse.bass as bass
    import concourse.tile as tile
    from concourse import mybir

    dt = mybir.dt.float32
    db = mybir.dt.bfloat16
    dh = mybir.dt.float16
    Alu = mybir.AluOpType
    Af = mybir.ActivationFunctionType

    nc = bacc.Bacc(
        "TRN2", target_bir_lowering=False, debug=False,
        enable_asserts=False, num_devices=NCORES,
    )
    sig = nc.dram_tensor("sig", [BPC, C, L], dh, kind="ExternalInput")
    pcols = nc.dram_tensor("pcols", [P, NPC], dt, kind="ExternalInput")
    eyew = nc.dram_tensor("eyew", [P, P], dh, kind="ExternalInput")
    diagw = nc.dram_tensor("diagw", [P, P], dh, kind="ExternalInput")
    pwt = nc.dram_tensor("pwt", [P, T0], dt, kind="ExternalInput")
    out = nc.dram_tensor("out", [BPC, C, L], dt, kind="ExternalOutput")

    BS = C * L          # batch stride (elems) in sig/out

    def src_ap(t, ch, lo, width):
        # outer dim = 32 time blocks so HWDGE spreads across all 16 DMA
        # engines (engine binding follows the outer dim); partition
        # p = tb*BPC + b
        return bass.AP(t, ch * L + lo, [[SPAN, TBS], [BS, BPC], [1, width]])

    def src_ap_b(t, b, ch, lo, width):
        # single-batch 2-dim AP: keeps the scheduler's DMA model accurate
        return bass.AP(t, b * BS + ch * L + lo, [[SPAN, TBS], [1, width]])

    with tile.TileContext(nc) as tc, ExitStack() as ctx:
        const = ctx.enter_context(tc.tile_pool(name="const", bufs=1))
        spool = ctx.enter_context(tc.tile_pool(name="sp", bufs=1))
        sqp = ctx.enter_context(tc.tile_pool(name="sq", bufs=2))
        wkp = ctx.enter_context(tc.tile_pool(name="wk", bufs=3))
        pse = ctx.enter_context(tc.tile_pool(name="pse", bufs=2, space="PSUM"))
        psh = ctx.enter_context(tc.tile_pool(name="psh", bufs=1, space="PSUM"))

        # ---- constants / params ----
        pc = const.tile([P, NPC], dt, tag="pc")
        eye = const.tile([P, P], dh, tag="eye")
        diag = const.tile([P, P], dh, tag="diag")
        pw = const.tile([P, T0], dt, tag="pw")
        epsc = const.tile([P, 1], dt, tag="epsc")
        ccol = const.tile([P, 1], dh, tag="ccol")

        def col(j):
            return pc[:, j:j + 1]

        # manual activation-table load: natural_log_exp_and_others (id 6)
        ld = mybir.InstLoadActFuncSet(
            name=nc.get_next_instruction_name(), act_func_set_id=6,
            ins=[], outs=[])
        ld.engine = mybir.EngineType.Activation
        nc.scalar.add_instruction(ld)

        nc.vector.memset(epsc, 1e-5)
        nc.vector.memset(ccol, 0.0)

        # ---- input DMAs: chunk 0 in halves for a fast start ----
        s = [spool.tile([P, C, CW], dh, tag=f"s{k}", name=f"s{k}")
             for k in range(NCH)]
        y = [spool.tile([P, CW], dh, tag=f"y{k}", name=f"y{k}")
             for k in range(NCH)]
        for ch in range(C):
            nc.sync.dma_start(s[0][:, ch, 0:HW], src_ap(sig, ch, 0, HW))
        nc.sync.dma_start(eye, eyew.ap())
        nc.sync.dma_start(pc, pcols.ap())
        for ch in range(C):
            nc.sync.dma_start(s[0][:, ch, HW:CW], src_ap(sig, ch, HW, HW))
        for ch in range(C):
            nc.sync.dma_start(s[1][:, ch, :], src_ap(sig, ch, CW, CW))
        nc.sync.dma_start(diag, diagw.ap())
        nc.sync.dma_start(pw, pwt.ap())
        for k in (2, 3):
            for ch in range(C):
                for b in range(BPC):
                    nc.sync.dma_start(s[k][b::BPC, ch, :],
                                      src_ap_b(sig, b, ch, k * CW, CW))

        def energy_half(k, j):
            """square (Act, bf16) -> PE adds into PSUM half -> scan."""
            lo = j * HW
            with tc.high_priority():
                sq = sqp.tile([P, C, HW], dh, tag="sq", name="sq")
                nc.scalar.activation(sq, s[k][:, :, lo:lo + HW], Af.Square)
                e_h = pse.tile([P, HW], dt, tag="e", name="e")
                for blk in range(0, HW, 512):
                    bs_ = slice(blk, blk + 512)
                    nc.tensor.matmul(e_h[:, bs_], eye, sq[:, 0, bs_],
                                     start=True, stop=False)
                    nc.tensor.matmul(e_h[:, bs_], eye, sq[:, 1, bs_],
                                     start=False, stop=True)
                if k == 0 and j == 0:
                    init = 0.0
                elif j == 0:
                    init = y[k - 1][:, CW - 1:CW]
                else:
                    init = y[k][:, lo - 1:lo]
                nc.vector.tensor_tensor_scan(
                    y[k][:, lo:lo + HW],
                    col(ALPHA).to_broadcast((P, HW)),
                    e_h, init, Alu.mult, Alu.add)

        # pieces: (chunk, lo, hi)
        pieces = [(0, HW // 2, CW), (1, 0, CW), (2, 0, CW),
                  (3, 0, CW), (0, 0, HW // 2)]
        # note HW//2 == 512 == T0MAX
        st = [None] * 5

        def p_ln(i):
            k, lo, hi = pieces[i]
            w = hi - lo
            x = wkp.tile([P, CW], dh, tag="x", name="x")
            with tc.high_priority():
                nc.scalar.activation(x[:, 0:w], y[k][:, lo:hi], Af.Ln,
                                     scale=col(ESCALE), bias=epsc[:, 0:1])
            st[i] = [x]

        def p_uvd(i):
            w = pieces[i][2] - pieces[i][1]
            x = st[i][0]
            uv = wkp.tile([P, CW], dh, tag="uv", name="uv")
            dti = wkp.tile([P, CW], dh, tag="d", name="d")
            xs, us, ds = x[:, 0:w], uv[:, 0:w], dti[:, 0:w]
            nc.vector.tensor_scalar(us, xs, col(UK), 0.0, Alu.add, Alu.max)
            nc.vector.tensor_scalar(ds, xs, col(UKK), 0.0, Alu.add, Alu.max)
            nc.vector.tensor_scalar(us, us, col(KNEE), col(RTA),
                                    Alu.min, Alu.mult)
            st[i] += [uv, dti]

        def p_t(i):
            w = pieces[i][2] - pieces[i][1]
            uv = st[i][1]
            tti = wkp.tile([P, CW], dh, tag="t", name="t")
            nc.scalar.activation(tti[:, 0:w], uv[:, 0:w], Af.Square)
            st[i].append(tti)

        def p_h(i):
            w = pieces[i][2] - pieces[i][1]
            _, _, dti, tti = st[i]
            h = psh.tile([P, CW], dt, tag="h", name="h")
            for blk in range(0, w, 512):
                bs_ = slice(blk, blk + 512)
                nc.tensor.matmul(h[:, bs_], eye, tti[:, bs_],
                                 start=True, stop=False)
                nc.tensor.matmul(h[:, bs_], diag, dti[:, bs_],
                                 start=False, stop=True)
            st[i].append(h)

        def p_exp(i):
            w = pieces[i][2] - pieces[i][1]
            uv, h = st[i][1], st[i][4]
            # gain overwrites uv (v' already consumed by t)
            nc.scalar.activation(uv[:, 0:w], h[:, 0:w], Af.Exp, scale=-1.0)

        def p_muls(i, eng0, eng1):
            k, lo, hi = pieces[i]
            w = hi - lo
            x, g = st[i][0], st[i][1]
            # ch0 -> dead x tile; ch1 -> dead y[k] slice (fp32)
            eng0.tensor_tensor(x[:, 0:w], g[:, 0:w],
                               s[k][:, 0, lo:hi], Alu.mult)
            eng1.tensor_tensor(y[k][:, lo:hi], g[:, 0:w],
                               s[k][:, 1, lo:hi], Alu.mult)
            nc.gpsimd.dma_start(src_ap(out, 0, k * CW + lo, w), x[:, 0:w])
            nc.gpsimd.dma_start(src_ap(out, 1, k * CW + lo, w),
                                y[k][:, lo:hi])

        # ---- schedule (hand-interleaved per-engine queues) ----
        for k in (0, 1):
            energy_half(k, 0)
            energy_half(k, 1)
        p_ln(0)                      # Act after sq c0/c1
        p_uvd(0)                     # DVE right after scan c1h1
        energy_half(2, 0)
        energy_half(2, 1)
        p_ln(1)
        p_t(0)
        p_h(0)
        p_uvd(1)                     # DVE before scan c3h0
        energy_half(3, 0)
        p_exp(0)
        energy_half(3, 1)
        p_t(1)
        p_h(1)
        # carry: C[p] = y3[p-BPC, CW-1] via partition-shift DMA, then fix
        nc.sync.dma_start(ccol[BPC:P, 0:1], y[3][0:P - BPC, CW - 1:CW])
        nc.vector.scalar_tensor_tensor(y[0][:, 0:T0], pw[:, 0:T0],
                                       ccol[:, 0:1], y[0][:, 0:T0],
                                       Alu.mult, Alu.add)
        p_exp(1)
        p_ln(2)
        p_uvd(2)
        p_t(2)
        p_h(2)
        p_ln(3)
        p_ln(4)
        p_exp(2)
        p_uvd(3)
        p_t(3)
        p_h(3)
        p_uvd(4)
        p_t(4)
        p_h(4)
        p_exp(3)
        p_exp(4)
        # all out-muls on DVE, serialized after the TS work: no engine
        # port contention (Pool TT degrades DVE 2x-mode ops 3-4x)
        for i in range(5):
            p_muls(i, nc.vector, nc.vector)

    nc.compile()
    return nc


def _get_program(T0):
    key = ("nc", T0)
    if key not in _cache:
        _cache[key] = _build_program(T0)
    return _cache[key]


def _run(inputs, trace=False):
    from concourse.bass_utils import run_bass_kernel_spmd

    sig_full = np.ascontiguousarray(
        np.asarray(inputs["input_signals"], np.float32).astype(np.float16))
    pv, negc1, alpha, T0 = _host_params(
        np.asarray(inputs["z_alpha_pre"], np.float32),
        np.asarray(inputs["log_threshold"], np.float32),
        np.asarray(inputs["log_ratio"], np.float32),
        np.asarray(inputs["log_knee"], np.float32),
    )
    nc = _get_program(T0)

    eye_b = np.eye(P, dtype=np.float16)
    tpow = np.arange(1, T0 + 1, dtype=np.float64)

    in_maps = []
    for c in range(NCORES):
        bsl = slice(c * BPC, (c + 1) * BPC)
        # per-partition param columns; partition p = tb*BPC + b
        cols = np.tile(pv[bsl], (TBS, 1))            # [128, NPC]
        dg = np.diag(np.tile(negc1[bsl], TBS)).astype(np.float32)
        pw_np = np.exp(tpow[None, :]
                       * np.log(np.tile(alpha[bsl], TBS))[:, None])
        pw_np[0:BPC, :] = 0.0    # tb=0 partitions take no cross-batch carry
        in_maps.append({
            "sig": np.ascontiguousarray(sig_full[bsl]),
            "pcols": np.ascontiguousarray(cols),
            "eyew": eye_b,
            "diagw": dg.astype(np.float16),
            "pwt": np.ascontiguousarray(pw_np.astype(np.float32)),
        })

    res = run_bass_kernel_spmd(
        nc, in_maps, core_ids=list(range(NCORES)), trace=trace,
    )
    outp = np.empty((N, C, L), dtype=np.float32)
    for c in range(NCORES):
        outp[c * BPC:(c + 1) * BPC] = res.results[c]["out"]
    return outp, res


def kernel(**inputs) -> np.ndarray:
    out, _ = _run(inputs, trace=False)
    return out


# revision 17
# speedup vs baseline: 1.1295x; 1.1295x over previous
"""Trainium2 Bass kernel for nn_BaseCompressor2 (truncated one-pole IIR
compressor), v2.

Mapping: 32 batches / 8 cores = 4 per core. Partition p = (b, tb) with
b = p>>5 the batch and tb = p&31 a time block; each partition owns 8192
consecutive samples, processed as 4 chunks of 2048 columns.

Per chunk: squares on Act (bf16 out) -> e = sq0+sq1 on the otherwise-idle
PE via bf16 identity matmuls accumulating in PSUM (half tiles, double
buffered) -> DVE tensor_tensor_scan (fp32 state) reads PSUM -> gain chain:
x = ln(escale*y + eps) [Act]; u = relu(x+uk), v' = sqrtA*min(u,knee),
d = relu(x+ukk) as DVE tensor_scalar 2x ops; t = v'^2 [Act];
h = I@t + diag(negc1)@d on PE (PSUM); gain = exp(-h) [Act];
out_c = gain*s_c on Pool (gpsimd). The piecewise knee is replaced by the
algebraically equivalent h = A*min(u,knee)^2 + negc1*relu(u-knee) (max
deviation |c1|*5e-4, far below tolerance).

Cross-partition IIR carry: C[p] = y[p-1, 8191] via a tiny SBUF->SBUF
partition-shift DMA after the last scan, applied as y0[:, :T0] += pw*C
with one small STT (alpha^(t+1) underflows beyond T0 <= 512). Chunk 0's
gain chain is split into cols [512:2048] (carry-free, runs early) and
[0:512] (after the fix, tiny tail).

A manual InstLoadActFuncSet(natural_log_exp_and_others) removes all
mid-kernel activation-table reloads.
"""

import numpy as np

N, C, L = 32, 2, 262144
NCORES = 8
BPC = N // NCORES          # batches per core
P = 128
TBS = P // BPC             # 32 time blocks per batch
SPAN = L // TBS            # 8192 cols per partition
NCH = 4                    # chunks per core
CW = SPAN // NCH           # 2048 chunk width
HW = CW // 2               # 1024 scan/e half width
T0MAX = 512

# pcols layout
ALPHA, ESCALE, UK, UKK, KNEE, RTA, NEGC1 = range(7)
NPC = 7

_cache = {}


def _host_params(z_alpha_pre, log_threshold, log_ratio, log_knee):
    z = z_alpha_pre.astype(np.float64).reshape(-1)
    thr = log_threshold.astype(np.float64).reshape(-1) - 6.0
    knee = np.exp(log_knee.astype(np.float64).reshape(-1))
    r001 = 1.0 + np.exp(log_ratio.astype(np.float64).reshape(-1)) + 0.001
    alpha = 1.0 / (1.0 + np.exp(-z))
    negc1 = 1.0 - 1.0 / r001
    # infinite-IIR == 16384-tap truncation in f32
    assert np.max(16384.0 * np.log(alpha)) < -60.0, "alpha too close to 1"
    lna_max = np.max(np.log(alpha))
    t0 = int(np.ceil(24.0 / max(1e-9, -lna_max))) + 24
    t0 = min(T0MAX, max(64, t0))
    assert (t0 + 1) * lna_max < -23.0, "carry horizon exceeds 512 cols"
    vals = np.zeros((N, NPC), dtype=np.float64)
    vals[:, ALPHA] = alpha
    vals[:, ESCALE] = (1.0 - alpha) / 2.0
    vals[:, UK] = knee / 2.0 - thr
    vals[:, UKK] = -knee / 2.0 - thr
    vals[:, KNEE] = knee
    vals[:, RTA] = np.sqrt(negc1 / (2.0 * (knee + 0.001)))
    vals[:, NEGC1] = negc1
    return vals.astype(np.float32), negc1, alpha, t0


def _build_program(T0):
    from contextlib import ExitStack

    import concourse.bacc as bacc
    import concourse.bass as bass
    import concourse.tile as tile
    from concourse import mybir

    dt = mybir.dt.float32
    db = mybir.dt.bfloat16
    dh = mybir.dt.float16
    Alu = mybir.AluOpType
    Af = mybir.ActivationFunctionType

    nc = bacc.Bacc(
        "TRN2", target_bir_lowering=False, debug=False,
        enable_asserts=False, num_devices=NCORES,
    )
    sig = nc.dram_tensor("sig", [BPC, C, L], dh, kind="ExternalInput")
    pcols = nc.dram_tensor("pcols", [P, NPC], dt, kind="ExternalInput")
    eyew = nc.dram_tensor("eyew", [P, P], dh, kind="ExternalInput")
    diagw = nc.dram_tensor("diagw", [P, P], dh, kind="ExternalInput")
    pwt = nc.dram_tensor("pwt", [P, T0], dt, kind="ExternalInput")
    out = nc.dram_tensor("out", [BPC, C, L], dt, kind="ExternalOutput")

    BS = C * L          # batch stride (elems) in sig/out

    def src_ap(t, ch, lo, width):
        # outer dim = 32 time blocks so HWDGE spreads across all 16 DMA
        # engines (engine binding follows the outer dim); partition
        # p = tb*BPC + b
        return bass.AP(t, ch * L + lo, [[SPAN, TBS], [BS, BPC], [1, width]])

    def src_ap_b(t, b, ch, lo, width):
        # single-batch 2-dim AP: keeps the scheduler's DMA model accurate
        return bass.AP(t, b * BS + ch * L + lo, [[SPAN, TBS], [1, width]])

    with tile.TileContext(nc) as tc, ExitStack() as ctx:
        const = ctx.enter_context(tc.tile_pool(name="const", bufs=1))
        spool = ctx.enter_context(tc.tile_pool(name="sp", bufs=1))
        sqp = ctx.enter_context(tc.tile_pool(name="sq", bufs=2))
        wkp = ctx.enter_context(tc.tile_pool(name="wk", bufs=3))
        pse = ctx.enter_context(tc.tile_pool(name="pse", bufs=2, space="PSUM"))
        psh = ctx.enter_context(tc.tile_pool(name="psh", bufs=1, space="PSUM"))

        # ---- constants / params ----
        pc = const.tile([P, NPC], dt, tag="pc")
        eye = const.tile([P, P], dh, tag="eye")
        diag = const.tile([P, P], dh, tag="diag")
        pw = const.tile([P, T0], dt, tag="pw")
        epsc = const.tile([P, 1], dt, tag="epsc")
        ccol = const.tile([P, 1], dh, tag="ccol")

        def col(j):
            return pc[:, j:j + 1]

        # manual activation-table load: natural_log_exp_and_others (id 6)
        ld = mybir.InstLoadActFuncSet(
            name=nc.get_next_instruction_name(), act_func_set_id=6,
            ins=[], outs=[])
        ld.engine = mybir.EngineType.Activation
        nc.scalar.add_instruction(ld)

        nc.vector.memset(epsc, 1e-5)
        nc.vector.memset(ccol, 0.0)

        # ---- input DMAs: chunk 0 in halves for a fast start ----
        s = [spool.tile([P, C, CW], dh, tag=f"s{k}", name=f"s{k}")
             for k in range(NCH)]
        y = [spool.tile([P, CW], dh, tag=f"y{k}", name=f"y{k}")
             for k in range(NCH)]
        for ch in range(C):
            nc.sync.dma_start(s[0][:, ch, 0:HW], src_ap(sig, ch, 0, HW))
        nc.sync.dma_start(eye, eyew.ap())
        nc.sync.dma_start(pc, pcols.ap())
        for ch in range(C):
            nc.sync.dma_start(s[0][:, ch, HW:CW], src_ap(sig, ch, HW, HW))
        for ch in range(C):
            nc.sync.dma_start(s[1][:, ch, :], src_ap(sig, ch, CW, CW))
        nc.sync.dma_start(diag, diagw.ap())
        nc.sync.dma_start(pw, pwt.ap())
        for k in (2, 3):
            for ch in range(C):
                nc.sync.dma_start(s[k][:, ch, :], src_ap(sig, ch, k * CW, CW))

        def energy_half(k, j):
            """square (Act, bf16) -> PE adds into PSUM half -> scan."""
            lo = j * HW
            with tc.high_priority():
                sq = sqp.tile([P, C, HW], dh, tag="sq", name="sq")
                nc.scalar.activation(sq, s[k][:, :, lo:lo + HW], Af.Square)
                e_h = pse.tile([P, HW], dt, tag="e", name="e")
                for blk in range(0, HW, 512):
                    bs_ = slice(blk, blk + 512)
                    nc.tensor.matmul(e_h[:, bs_], eye, sq[:, 0, bs_],
                                     start=True, stop=False)
                    nc.tensor.matmul(e_h[:, bs_], eye, sq[:, 1, bs_],
                                     start=False, stop=True)
                if k == 0 and j == 0:
                    init = 0.0
                elif j == 0:
                    init = y[k - 1][:, CW - 1:CW]
                else:
                    init = y[k][:, lo - 1:lo]
                nc.vector.tensor_tensor_scan(
                    y[k][:, lo:lo + HW],
                    col(ALPHA).to_broadcast((P, HW)),
                    e_h, init, Alu.mult, Alu.add)

        # pieces: (chunk, lo, hi)
        pieces = [(0, HW // 2, CW), (1, 0, CW), (2, 0, CW),
                  (3, 0, CW), (0, 0, HW // 2)]
        # note HW//2 == 512 == T0MAX
        st = [None] * 5

        def p_ln(i):
            k, lo, hi = pieces[i]
            w = hi - lo
            x = wkp.tile([P, CW], dh, tag="x", name="x")
            with tc.high_priority():
                nc.scalar.activation(x[:, 0:w], y[k][:, lo:hi], Af.Ln,
                                     scale=col(ESCALE), bias=epsc[:, 0:1])
            st[i] = [x]

        def p_uvd(i):
            w = pieces[i][2] - pieces[i][1]
            x = st[i][0]
            uv = wkp.tile([P, CW], dh, tag="uv", name="uv")
            dti = wkp.tile([P, CW], dh, tag="d", name="d")
            xs, us, ds = x[:, 0:w], uv[:, 0:w], dti[:, 0:w]
            nc.vector.tensor_scalar(us, xs, col(UK), 0.0, Alu.add, Alu.max)
            nc.vector.tensor_scalar(ds, xs, col(UKK), 0.0, Alu.add, Alu.max)
            nc.vector.tensor_scalar(us, us, col(KNEE), col(RTA),
                                    Alu.min, Alu.mult)
            st[i] += [uv, dti]

        def p_t(i):
            w = pieces[i][2] - pieces[i][1]
            uv = st[i][1]
            tti = wkp.tile([P, CW], dh, tag="t", name="t")
            nc.scalar.activation(tti[:, 0:w], uv[:, 0:w], Af.Square)
            st[i].append(tti)

        def p_h(i):
            w = pieces[i][2] - pieces[i][1]
            _, _, dti, tti = st[i]
            h = psh.tile([P, CW], dt, tag="h", name="h")
            for blk in range(0, w, 512):
                bs_ = slice(blk, blk + 512)
                nc.tensor.matmul(h[:, bs_], eye, tti[:, bs_],
                                 start=True, stop=False)
                nc.tensor.matmul(h[:, bs_], diag, dti[:, bs_],
                                 start=False, stop=True)
            st[i].append(h)

        def p_exp(i):
            w = pieces[i][2] - pieces[i][1]
            uv, h = st[i][1], st[i][4]
            # gain overwrites uv (v' already consumed by t)
            nc.scalar.activation(uv[:, 0:w], h[:, 0:w], Af.Exp, scale=-1.0)

        def p_muls(i, eng0, eng1):
            k, lo, hi = pieces[i]
            w = hi - lo
            x, g = st[i][0], st[i][1]
            # ch0 -> dead x tile; ch1 -> dead y[k] slice (fp32)
            eng0.tensor_tensor(x[:, 0:w], g[:, 0:w],
                               s[k][:, 0, lo:hi], Alu.mult)
            eng1.tensor_tensor(y[k][:, lo:hi], g[:, 0:w],
                               s[k][:, 1, lo:hi], Alu.mult)
            nc.gpsimd.dma_start(src_ap(out, 0, k * CW + lo, w), x[:, 0:w])
            nc.gpsimd.dma_start(src_ap(out, 1, k * CW + lo, w),
                                y[k][:, lo:hi])

        # ---- schedule (hand-interleaved per-engine queues) ----
        for k in (0, 1):
            energy_half(k, 0)
            energy_half(k, 1)
        p_ln(0)                      # Act after sq c0/c1
        p_uvd(0)                     # DVE right after scan c1h1
        energy_half(2, 0)
        energy_half(2, 1)
        p_ln(1)
        p_t(0)
        p_h(0)
        p_uvd(1)                     # DVE before scan c3h0
        energy_half(3, 0)
        p_exp(0)
        energy_half(3, 1)
        p_t(1)
        p_h(1)
        # carry: C[p] = y3[p-BPC, CW-1] via partition-shift DMA, then fix
        nc.sync.dma_start(ccol[BPC:P, 0:1], y[3][0:P - BPC, CW - 1:CW])
        nc.vector.scalar_tensor_tensor(y[0][:, 0:T0], pw[:, 0:T0],
                                       ccol[:, 0:1], y[0][:, 0:T0],
                                       Alu.mult, Alu.add)
        p_exp(1)
        p_ln(2)
        p_uvd(2)
        p_t(2)
        p_h(2)
        p_ln(3)
        p_ln(4)
        p_exp(2)
        p_uvd(3)
        p_t(3)
        p_h(3)
        p_uvd(4)
        p_t(4)
        p_h(4)
        p_exp(3)
        p_exp(4)
        # all out-muls on DVE, serialized after the TS work: no engine
        # port contention (Pool TT degrades DVE 2x-mode ops 3-4x)
        for i in range(5):
            p_muls(i, nc.vector, nc.vector)

    nc.compile()
    return nc


def _get_program(T0):
    key = ("nc", T0)
    if key not in _cache:
        _cache[key] = _build_program(T0)
    return _cache[key]


def _run(inputs, trace=False):
    from concourse.bass_utils import run_bass_kernel_spmd

    sig_full = np.ascontiguousarray(
        np.asarray(inputs["input_signals"], np.float32).astype(np.float16))
    pv, negc1, alpha, T0 = _host_params(
        np.asarray(inputs["z_alpha_pre"], np.float32),
        np.asarray(inputs["log_threshold"], np.float32),
        np.asarray(inputs["log_ratio"], np.float32),
        np.asarray(inputs["log_knee"], np.float32),
    )
    nc = _get_program(T0)

    eye_b = np.eye(P, dtype=np.float16)
    tpow = np.arange(1, T0 + 1, dtype=np.float64)

    in_maps = []
    for c in range(NCORES):
        bsl = slice(c * BPC, (c + 1) * BPC)
        # per-partition param columns; partition p = tb*BPC + b
        cols = np.tile(pv[bsl], (TBS, 1))            # [128, NPC]
        dg = np.diag(np.tile(negc1[bsl], TBS)).astype(np.float32)
        pw_np = np.exp(tpow[None, :]
                       * np.log(np.tile(alpha[bsl], TBS))[:, None])
        pw_np[0:BPC, :] = 0.0    # tb=0 partitions take no cross-batch carry
        in_maps.append({
            "sig": np.ascontiguousarray(sig_full[bsl]),
            "pcols": np.ascontiguousarray(cols),
            "eyew": eye_b,
            "diagw": dg.astype(np.float16),
            "pwt": np.ascontiguousarray(pw_np.astype(np.float32)),
        })

    res = run_bass_kernel_spmd(
        nc, in_maps, core_ids=list(range(NCORES)), trace=trace,
    )
    outp = np.empty((N, C, L), dtype=np.float32)
    for c in range(NCORES):
        outp[c * BPC:(c + 1) * BPC] = res.results[c]["out"]
    return outp, res


def kernel(**inputs) -> np.ndarray:
    out, _ = _run(inputs, trace=False)
    return out


# revision 18
# speedup vs baseline: 1.3054x; 1.1557x over previous
"""Trainium2 Bass kernel for nn_BaseCompressor2 (truncated one-pole IIR
compressor), v2.

Mapping: 32 batches / 8 cores = 4 per core. Partition p = (b, tb) with
b = p>>5 the batch and tb = p&31 a time block; each partition owns 8192
consecutive samples, processed as 4 chunks of 2048 columns.

Per chunk: squares on Act (bf16 out) -> e = sq0+sq1 on the otherwise-idle
PE via bf16 identity matmuls accumulating in PSUM (half tiles, double
buffered) -> DVE tensor_tensor_scan (fp32 state) reads PSUM -> gain chain:
x = ln(escale*y + eps) [Act]; u = relu(x+uk), v' = sqrtA*min(u,knee),
d = relu(x+ukk) as DVE tensor_scalar 2x ops; t = v'^2 [Act];
h = I@t + diag(negc1)@d on PE (PSUM); gain = exp(-h) [Act];
out_c = gain*s_c on Pool (gpsimd). The piecewise knee is replaced by the
algebraically equivalent h = A*min(u,knee)^2 + negc1*relu(u-knee) (max
deviation |c1|*5e-4, far below tolerance).

Cross-partition IIR carry: C[p] = y[p-1, 8191] via a tiny SBUF->SBUF
partition-shift DMA after the last scan, applied as y0[:, :T0] += pw*C
with one small STT (alpha^(t+1) underflows beyond T0 <= 512). Chunk 0's
gain chain is split into cols [512:2048] (carry-free, runs early) and
[0:512] (after the fix, tiny tail).

A manual InstLoadActFuncSet(natural_log_exp_and_others) removes all
mid-kernel activation-table reloads.
"""

import numpy as np

N, C, L = 32, 2, 262144
NCORES = 8
BPC = N // NCORES          # batches per core
P = 128
TBS = P // BPC             # 32 time blocks per batch
SPAN = L // TBS            # 8192 cols per partition
NCH = 4                    # chunks per core
CW = SPAN // NCH           # 2048 chunk width
HW = CW // 2               # 1024 scan/e half width
T0MAX = 512

# pcols layout
ALPHA, ESCALE, UK, UKK, KNEE, RTA, NEGC1 = range(7)
NPC = 7

_cache = {}


def _host_params(z_alpha_pre, log_threshold, log_ratio, log_knee):
    z = z_alpha_pre.astype(np.float64).reshape(-1)
    thr = log_threshold.astype(np.float64).reshape(-1) - 6.0
    knee = np.exp(log_knee.astype(np.float64).reshape(-1))
    r001 = 1.0 + np.exp(log_ratio.astype(np.float64).reshape(-1)) + 0.001
    alpha = 1.0 / (1.0 + np.exp(-z))
    negc1 = 1.0 - 1.0 / r001
    # infinite-IIR == 16384-tap truncation in f32
    assert np.max(16384.0 * np.log(alpha)) < -60.0, "alpha too close to 1"
    lna_max = np.max(np.log(alpha))
    t0 = int(np.ceil(24.0 / max(1e-9, -lna_max))) + 24
    t0 = min(T0MAX, max(64, t0))
    assert (t0 + 1) * lna_max < -23.0, "carry horizon exceeds 512 cols"
    vals = np.zeros((N, NPC), dtype=np.float64)
    vals[:, ALPHA] = alpha
    vals[:, ESCALE] = (1.0 - alpha) / 2.0
    vals[:, UK] = knee / 2.0 - thr
    vals[:, UKK] = -knee / 2.0 - thr
    vals[:, KNEE] = knee
    vals[:, RTA] = np.sqrt(negc1 / (2.0 * (knee + 0.001)))
    vals[:, NEGC1] = negc1
    return vals.astype(np.float32), negc1, alpha, t0


def _build_program(T0):
    from contextlib import ExitStack

    import concourse.bacc as bacc
    import concourse.bass as bass
    import concourse.tile as tile
    from concourse import mybir

    dt = mybir.dt.float32
    db = mybir.dt.bfloat16
    dh = mybir.dt.float16
    Alu = mybir.AluOpType
    Af = mybir.ActivationFunctionType

    nc = bacc.Bacc(
        "TRN2", target_bir_lowering=False, debug=False,
        enable_asserts=False, num_devices=NCORES,
    )
    sig = nc.dram_tensor("sig", [BPC, C, L], dh, kind="ExternalInput")
    pcols = nc.dram_tensor("pcols", [P, NPC], dt, kind="ExternalInput")
    eyew = nc.dram_tensor("eyew", [P, P], dh, kind="ExternalInput")
    diagw = nc.dram_tensor("diagw", [P, P], dh, kind="ExternalInput")
    pwt = nc.dram_tensor("pwt", [P, T0], dt, kind="ExternalInput")
    out = nc.dram_tensor("out", [BPC, C, L], dt, kind="ExternalOutput")

    BS = C * L          # batch stride (elems) in sig/out

    def src_ap(t, ch, lo, width):
        # outer dim = 32 time blocks so HWDGE spreads across all 16 DMA
        # engines (engine binding follows the outer dim); partition
        # p = tb*BPC + b
        return bass.AP(t, ch * L + lo, [[SPAN, TBS], [BS, BPC], [1, width]])

    def src_ap_b(t, b, ch, lo, width):
        # single-batch 2-dim AP: keeps the scheduler's DMA model accurate
        return bass.AP(t, b * BS + ch * L + lo, [[SPAN, TBS], [1, width]])

    with tile.TileContext(nc) as tc, ExitStack() as ctx:
        const = ctx.enter_context(tc.tile_pool(name="const", bufs=1))
        spool = ctx.enter_context(tc.tile_pool(name="sp", bufs=1))
        sqp = ctx.enter_context(tc.tile_pool(name="sq", bufs=2))
        wkp = ctx.enter_context(tc.tile_pool(name="wk", bufs=3))
        pse = ctx.enter_context(tc.tile_pool(name="pse", bufs=2, space="PSUM"))
        psh = ctx.enter_context(tc.tile_pool(name="psh", bufs=1, space="PSUM"))

        # ---- constants / params ----
        pc = const.tile([P, NPC], dt, tag="pc")
        eye = const.tile([P, P], dh, tag="eye")
        diag = const.tile([P, P], dh, tag="diag")
        pw = const.tile([P, T0], dt, tag="pw")
        epsc = const.tile([P, 1], dt, tag="epsc")
        ccol = const.tile([P, 1], dh, tag="ccol")

        def col(j):
            return pc[:, j:j + 1]

        # manual activation-table load: natural_log_exp_and_others (id 6)
        ld = mybir.InstLoadActFuncSet(
            name=nc.get_next_instruction_name(), act_func_set_id=6,
            ins=[], outs=[])
        ld.engine = mybir.EngineType.Activation
        nc.scalar.add_instruction(ld)

        nc.vector.memset(epsc, 1e-5)
        nc.vector.memset(ccol, 0.0)

        # ---- input DMAs: chunk 0 in halves for a fast start ----
        s = [spool.tile([P, C, CW], dh, tag=f"s{k}", name=f"s{k}")
             for k in range(NCH)]
        y = [spool.tile([P, CW], dh, tag=f"y{k}", name=f"y{k}")
             for k in range(NCH)]
        for ch in range(C):
            nc.sync.dma_start(s[0][:, ch, 0:HW], src_ap(sig, ch, 0, HW))
        nc.sync.dma_start(eye, eyew.ap())
        nc.sync.dma_start(pc, pcols.ap())
        for ch in range(C):
            nc.sync.dma_start(s[0][:, ch, HW:CW], src_ap(sig, ch, HW, HW))
        for j in (0, 1):
            for ch in range(C):
                nc.sync.dma_start(s[1][:, ch, j * HW:(j + 1) * HW],
                                  src_ap(sig, ch, CW + j * HW, HW))
        nc.sync.dma_start(diag, diagw.ap())
        nc.sync.dma_start(pw, pwt.ap())
        for k in (2, 3):
            for j in (0, 1):
                for ch in range(C):
                    nc.sync.dma_start(s[k][:, ch, j * HW:(j + 1) * HW],
                                      src_ap(sig, ch, k * CW + j * HW, HW))

        def energy_half(k, j):
            """square (Act, bf16) -> PE adds into PSUM half -> scan."""
            lo = j * HW
            with tc.high_priority():
                sq = sqp.tile([P, C, HW], dh, tag="sq", name="sq")
                nc.scalar.activation(sq, s[k][:, :, lo:lo + HW], Af.Square)
                e_h = pse.tile([P, HW], dt, tag="e", name="e")
                for blk in range(0, HW, 512):
                    bs_ = slice(blk, blk + 512)
                    nc.tensor.matmul(e_h[:, bs_], eye, sq[:, 0, bs_],
                                     start=True, stop=False)
                    nc.tensor.matmul(e_h[:, bs_], eye, sq[:, 1, bs_],
                                     start=False, stop=True)
                if k == 0 and j == 0:
                    init = 0.0
                elif j == 0:
                    init = y[k - 1][:, CW - 1:CW]
                else:
                    init = y[k][:, lo - 1:lo]
                nc.vector.tensor_tensor_scan(
                    y[k][:, lo:lo + HW],
                    col(ALPHA).to_broadcast((P, HW)),
                    e_h, init, Alu.mult, Alu.add)

        # pieces: (chunk, lo, hi)
        pieces = [(0, HW // 2, CW), (1, 0, CW), (2, 0, CW),
                  (3, 0, CW), (0, 0, HW // 2)]
        # note HW//2 == 512 == T0MAX
        st = [None] * 5

        def p_ln(i):
            k, lo, hi = pieces[i]
            w = hi - lo
            x = wkp.tile([P, CW], dh, tag="x", name="x")
            with tc.high_priority():
                nc.scalar.activation(x[:, 0:w], y[k][:, lo:hi], Af.Ln,
                                     scale=col(ESCALE), bias=epsc[:, 0:1])
            st[i] = [x]

        def p_uvd(i):
            w = pieces[i][2] - pieces[i][1]
            x = st[i][0]
            uv = wkp.tile([P, CW], dh, tag="uv", name="uv")
            dti = wkp.tile([P, CW], dh, tag="d", name="d")
            xs, us, ds = x[:, 0:w], uv[:, 0:w], dti[:, 0:w]
            nc.vector.tensor_scalar(us, xs, col(UK), 0.0, Alu.add, Alu.max)
            nc.vector.tensor_scalar(ds, xs, col(UKK), 0.0, Alu.add, Alu.max)
            nc.vector.tensor_scalar(us, us, col(KNEE), col(RTA),
                                    Alu.min, Alu.mult)
            st[i] += [uv, dti]

        def p_t(i):
            w = pieces[i][2] - pieces[i][1]
            uv = st[i][1]
            tti = wkp.tile([P, CW], dh, tag="t", name="t")
            nc.vector.tensor_tensor(tti[:, 0:w], uv[:, 0:w], uv[:, 0:w],
                                    Alu.mult)
            st[i].append(tti)

        def p_h(i):
            w = pieces[i][2] - pieces[i][1]
            _, _, dti, tti = st[i]
            h = psh.tile([P, CW], dt, tag="h", name="h")
            for blk in range(0, w, 512):
                bs_ = slice(blk, blk + 512)
                nc.tensor.matmul(h[:, bs_], eye, tti[:, bs_],
                                 start=True, stop=False)
                nc.tensor.matmul(h[:, bs_], diag, dti[:, bs_],
                                 start=False, stop=True)
            st[i].append(h)

        def p_exp(i):
            w = pieces[i][2] - pieces[i][1]
            uv, h = st[i][1], st[i][4]
            # gain overwrites uv (v' already consumed by t)
            nc.scalar.activation(uv[:, 0:w], h[:, 0:w], Af.Exp, scale=-1.0)

        def p_muls(i, eng0, eng1):
            k, lo, hi = pieces[i]
            w = hi - lo
            x, g = st[i][0], st[i][1]
            # ch0 -> dead x tile; ch1 -> dead y[k] slice (fp32)
            eng0.tensor_tensor(x[:, 0:w], g[:, 0:w],
                               s[k][:, 0, lo:hi], Alu.mult)
            eng1.tensor_tensor(y[k][:, lo:hi], g[:, 0:w],
                               s[k][:, 1, lo:hi], Alu.mult)
            nc.gpsimd.dma_start(src_ap(out, 0, k * CW + lo, w), x[:, 0:w])
            nc.gpsimd.dma_start(src_ap(out, 1, k * CW + lo, w),
                                y[k][:, lo:hi])

        # ---- schedule (hand-interleaved per-engine queues) ----
        for k in (0, 1):
            energy_half(k, 0)
            energy_half(k, 1)
        p_ln(0)                      # Act after sq c0/c1
        p_uvd(0)                     # DVE right after scan c1h1
        energy_half(2, 0)
        energy_half(2, 1)
        p_ln(1)
        p_t(0)
        p_h(0)
        p_uvd(1)                     # DVE before scan c3h0
        energy_half(3, 0)
        p_exp(0)
        energy_half(3, 1)
        p_t(1)
        p_h(1)
        # carry: C[p] = y3[p-BPC, CW-1] via partition-shift DMA, then fix
        nc.sync.dma_start(ccol[BPC:P, 0:1], y[3][0:P - BPC, CW - 1:CW])
        nc.vector.scalar_tensor_tensor(y[0][:, 0:T0], pw[:, 0:T0],
                                       ccol[:, 0:1], y[0][:, 0:T0],
                                       Alu.mult, Alu.add)
        p_exp(1)
        p_ln(2)
        p_uvd(2)
        p_t(2)
        p_h(2)
        p_ln(3)
        p_ln(4)
        p_exp(2)
        p_uvd(3)
        p_t(3)
        p_h(3)
        p_uvd(4)
        p_t(4)
        p_h(4)
        p_exp(3)
        p_exp(4)
        # all out-muls on DVE, serialized after the TS work: no engine
        # port contention (Pool TT degrades DVE 2x-mode ops 3-4x)
        for i in range(5):
            p_muls(i, nc.vector, nc.vector)

    nc.compile()
    return nc


def _get_program(T0):
    key = ("nc", T0)
    if key not in _cache:
        _cache[key] = _build_program(T0)
    return _cache[key]


def _run(inputs, trace=False):
    from concourse.bass_utils import run_bass_kernel_spmd

    sig_full = np.ascontiguousarray(
        np.asarray(inputs["input_signals"], np.float32).astype(np.float16))
    pv, negc1, alpha, T0 = _host_params(
        np.asarray(inputs["z_alpha_pre"], np.float32),
        np.asarray(inputs["log_threshold"], np.float32),
        np.asarray(inputs["log_ratio"], np.float32),
        np.asarray(inputs["log_knee"], np.float32),
    )
    nc = _get_program(T0)

    eye_b = np.eye(P, dtype=np.float16)
    tpow = np.arange(1, T0 + 1, dtype=np.float64)

    in_maps = []
    for c in range(NCORES):
        bsl = slice(c * BPC, (c + 1) * BPC)
        # per-partition param columns; partition p = tb*BPC + b
        cols = np.tile(pv[bsl], (TBS, 1))            # [128, NPC]
        dg = np.diag(np.tile(negc1[bsl], TBS)).astype(np.float32)
        pw_np = np.exp(tpow[None, :]
                       * np.log(np.tile(alpha[bsl], TBS))[:, None])
        pw_np[0:BPC, :] = 0.0    # tb=0 partitions take no cross-batch carry
        in_maps.append({
            "sig": np.ascontiguousarray(sig_full[bsl]),
            "pcols": np.ascontiguousarray(cols),
            "eyew": eye_b,
            "diagw": dg.astype(np.float16),
            "pwt": np.ascontiguousarray(pw_np.astype(np.float32)),
        })

    res = run_bass_kernel_spmd(
        nc, in_maps, core_ids=list(range(NCORES)), trace=trace,
    )
    outp = np.empty((N, C, L), dtype=np.float32)
    for c in range(NCORES):
        outp[c * BPC:(c + 1) * BPC] = res.results[c]["out"]
    return outp, res


def kernel(**inputs) -> np.ndarray:
    out, _ = _run(inputs, trace=False)
    return out
